# revision 1
# baseline (speedup 1.0000x reference)
"""Tensor-parallel LlamaAttention (B=1, S=2048, H=4096, 32 q-heads / 8 kv-heads,
head_dim=128) on 8 Trainium2 NeuronCores.

Sharding: core c owns query heads 4c..4c+3 and KV head c (GQA group), i.e.
Wq rows [512c, 512c+512), Wk/Wv rows [128c, 128c+128), and Wo columns
[512c, 512c+512). Each core produces a full-shape [2048, 4096] partial of the
output projection; the host sums the 8 partials.

All device-side matmuls run in "transposed" layouts so no large on-device
transposes are needed:
  - scores are computed as S.T[k, q] (k on partitions) so softmax needs no
    row-max (scores are O(1), exp cannot overflow) and the denominator is a
    ones-vector matmul over the partition dim.
  - attention output comes out as attnT[d, q], which is exactly the stationary
    operand layout the output projection needs.
"""

import math
import sys

sys.path.insert(0, "/opt/trn_rl_repo")

import numpy as np

import concourse.bass as bass
import concourse.mybir as mybir
import concourse.tile as tile_mod
from concourse.tile import ScopedClock

F32 = mybir.dt.float32
F32R = mybir.dt.float32r


S = 2048
H = 4096
DQ = 512  # per-core query width (4 heads x 128)
DKV = 128  # per-core kv width (1 head)
D = 128  # head dim
N_CORES = 8
HEADS = 4  # q heads per core
ROPE_THETA = 500000.0
SM_SCALE = 1.0 / math.sqrt(D)

HT = H // 128  # 32 contraction tiles
ST_A = 512  # pass-A moving-operand width
N_ST_A = S // ST_A
HQ = 8  # h-tiles per X chunk (1024 rows)
N_HQ = HT // HQ
QT_W = 512  # phase-B q-tile width
N_QT = S // QT_W
N_KT = S // 128  # 16 k-tiles of 128
ET = 512  # phase-C output e-tile width


def _patch_tilecontext():
    """walrus's CTRL codegen rejects >2 sync waits on one instruction; the
    Tile kernel-tail drain waits on the whole global clock. Spread the waits
    one-per-nop before the drain."""
    if getattr(tile_mod.TileContext, "_drain_patched", False):
        return

    def _drain_and_barrier(self, tick_clock, wait_clock):
        nc = self.nc
        probe = nc.sync.nop(nofuse=True)
        wait_clock.add_sem_waits(
            probe.ins, ScopedClock({None: tick_clock.global_clock})
        )
        si = probe.ins.sync_info
        waits = list(si.on_wait or [])
        if len(waits) > 1:
            si.on_wait = waits[:1]
            for w in waits[1:]:
                n = nc.sync.nop(nofuse=True)
                if n.ins.sync_info is None:
                    n.ins.sync_info = mybir.SyncInfo(on_wait=[w], on_update=[])
                else:
                    n.ins.sync_info.on_wait = [w]
        nc.sync.drain()
        nc.all_engine_barrier()
        assert self.sems is not None
        popped = nc._tile_sem_poison_stack.pop()
        assert popped is self._sem_poison
        nc.clear_and_free_semaphores(list(self.sems.allocated().values()))
        nc.all_engine_barrier()

    tile_mod.TileContext._drain_and_barrier = _drain_and_barrier
    tile_mod.TileContext._drain_patched = True


def _split_sync_waits(nc, cap=1):
    """walrus's CoreV3 codegen rejects instructions carrying more than ~2
    sync-wait commands. Hoist extra waits onto nops inserted just before the
    instruction on the same engine (sound: Tile data-dep waits are
    sem-ge-imm, i.e. monotone)."""
    n_split = 0
    for fn in nc.m.functions:
        for bb in fn.blocks:
            new_insts = []
            for inst in bb.instructions:
                si = inst.sync_info
                waits = list(si.on_wait) if si and si.on_wait else []
                if len(waits) > cap:
                    keep = waits[-cap:]
                    for j, w in enumerate(waits[:-cap]):
                        nop = mybir.InstNoOp(
                            name=f"{inst.name}-wsplit{j}", ins=[], outs=[]
                        )
                        nop.engine = inst.engine
                        nop.sync_info = mybir.SyncInfo(on_wait=[w], on_update=[])
                        new_insts.append(nop)
                        n_split += 1
                    si.on_wait = keep
                new_insts.append(inst)
            bb.instructions[:] = new_insts
    return n_split


def _rope_epilogue(nc, pool, ps, out_ap, cos_ap, sin_ap, width):
    """out = ps * cos + rotate_half(ps) * sin_signed, straight out of PSUM.

    sin_ap carries the sign fold: rows 0:64 hold -sin, rows 64:128 hold +sin,
    so rotate_half is just a 64-partition swap on the ps read."""
    t1 = pool.tile([128, width], F32, tag="rope_t1")
    t2 = pool.tile([128, width], F32, tag="rope_t2")
    nc.vector.tensor_mul(t1[:], ps[:], cos_ap)
    nc.vector.tensor_mul(t2[0:64, :], ps[64:128, :], sin_ap[0:64, :])
    nc.vector.tensor_mul(t2[64:128, :], ps[0:64, :], sin_ap[64:128, :])
    nc.vector.tensor_add(out_ap, t1[:], t2[:])


def _build_program():
    _patch_tilecontext()
    nc = bass.Bass()

    xT = nc.declare_dram_parameter("xT", [H, S], F32R, isOutput=False)
    wqT = nc.declare_dram_parameter("wqT", [H, DQ], F32R, isOutput=False)
    wkT = nc.declare_dram_parameter("wkT", [H, DKV], F32R, isOutput=False)
    wvT = nc.declare_dram_parameter("wvT", [H, DKV], F32R, isOutput=False)
    woT = nc.declare_dram_parameter("woT", [DQ, H], F32R, isOutput=False)
    cosT = nc.declare_dram_parameter("cosT", [D, S], F32, isOutput=False)
    sinT = nc.declare_dram_parameter("sinT", [D, S], F32, isOutput=False)
    masks = nc.declare_dram_parameter("masks", [128, 4 * QT_W], F32R, isOutput=False)
    ident = nc.declare_dram_parameter("ident", [128, 128], F32, isOutput=False)
    ones = nc.declare_dram_parameter("ones", [128, 128], F32R, isOutput=False)
    out = nc.declare_dram_parameter("out", [S, H], F32, isOutput=True)

    qT_dram = nc.dram_tensor("qT_scratch", [HEADS, D, S], F32R)

    xT_t = xT[:].rearrange("(ht p) s -> p ht s", p=128)
    wqT_t = wqT[:].rearrange("(ht p) d -> p ht d", p=128)
    wkT_t = wkT[:].rearrange("(ht p) d -> p ht d", p=128)
    wvT_t = wvT[:].rearrange("(ht p) d -> p ht d", p=128)
    woT_t = woT[:].rearrange("(j p) e -> p j e", p=128)

    from contextlib import ExitStack

    with tile_mod.TileContext(nc) as tc:
        with ExitStack() as _stk:
            persist = _stk.enter_context(tc.tile_pool(name="persist", bufs=1))
            kt_sb = persist.tile([128, S], F32R)  # K.T, rope'd (d x k)
            v_sb = persist.tile([128, N_KT, 128], F32R)  # V natural (k x d) tiles
            ones_sb = persist.tile([128, 128], F32R)
            nc.scalar.dma_start(out=ones_sb[:], in_=ones[:])

            b_qt = _stk.enter_context(tc.tile_pool(name="b_qt", bufs=6))

            with tc.tile_pool(name="cs", bufs=1) as cs:
                cos_sb = cs.tile([128, S], F32, tag="cos")
                sin_sb = cs.tile([128, S], F32, tag="sin")

                # ---- Pass A: all QKV projections in one X sweep. Per s-tile,
                # 6 PSUM banks accumulate k/v/q0..q3 over H, with X streamed
                # in h-quarters so the working tile stays small. KV weights
                # (small DMAs, ACT queue) land first so PE starts early; X
                # streams on the SP queue in parallel.
                with tc.tile_pool(name="aw", bufs=1) as aw, \
                     tc.tile_pool(name="a_xt", bufs=2) as a_xt, \
                     tc.tile_pool(name="a_st", bufs=3) as a_st, \
                     tc.tile_pool(name="a_ps", bufs=1, space="PSUM") as a_ps, \
                     tc.tile_pool(name="a_tps", bufs=2, space="PSUM") as a_tps:
                    # DMA emission order sets queue-drain order: the first X
                    # chunk and the KV weights gate the first matmuls, so
                    # they go first; cos/sin aren't needed until the first
                    # epilogue ~30us in.
                    xt00 = a_xt.tile([128, HQ // 2, ST_A], F32R, tag="xt",
                                     name="xt00")
                    nc.sync.dma_start(
                        out=xt00[:], in_=xT_t[:, 0 : HQ // 2, 0:ST_A]
                    )
                    wk_sb = aw.tile([128, HT, DKV], F32R, tag="wk")
                    wv_sb = aw.tile([128, HT, DKV], F32R, tag="wv")
                    nc.scalar.dma_start(out=wk_sb[:], in_=wkT_t)
                    nc.scalar.dma_start(out=wv_sb[:], in_=wvT_t)
                    wq_sb = aw.tile([128, HT, DQ], F32R, tag="wq")
                    for h in range(HEADS):  # per-head chunks: q0 lands first
                        nc.scalar.dma_start(
                            out=wq_sb[:, :, bass.ts(h, D)],
                            in_=wqT_t[:, :, bass.ts(h, D)],
                        )
                    nc.scalar.dma_start(out=cos_sb[:], in_=cosT[:])
                    nc.scalar.dma_start(out=sin_sb[:], in_=sinT[:])
                    ident_sb = aw.tile([128, 128], F32, tag="ident")
                    nc.scalar.dma_start(out=ident_sb[:], in_=ident[:])
                    vt_sb = aw.tile([128, S], F32, tag="vt")  # V.T staging

                    # d-tile list: (lhsT 3d tile, d-slice, kind); kv first
                    dlist = [(wk_sb, slice(0, D), "k"), (wv_sb, slice(0, D), "v")]
                    dlist += [(wq_sb, bass.ts(h, D), f"q{h}") for h in range(HEADS)]

                    for st in range(N_ST_A):
                        ssl = bass.ts(st, ST_A)
                        ps_tiles = {}
                        for _, _, kind in dlist:
                            ps_tiles[kind] = a_ps.tile(
                                [128, ST_A], F32, tag=f"mm_{kind}",
                                name=f"ps_{kind}_{st}",
                            )
                        hq = HQ // 2 if st == 0 else HQ
                        for hh in range(HT // hq):
                            if st == 0 and hh == 0:
                                xt = xt00
                            else:
                                xt = a_xt.tile([128, hq, ST_A], F32R,
                                               tag="xt", name=f"xt_{st}_{hh}")
                                xt_eng = nc.sync if hh % 2 == 0 else nc.gpsimd
                                xt_eng.dma_start(
                                    out=xt[:],
                                    in_=xT_t[:, hh * hq : (hh + 1) * hq, ssl],
                                )
                            for w_sb, dsl, kind in dlist:
                                ps = ps_tiles[kind]
                                for ht in range(hq):
                                    nc.tensor.matmul(
                                        ps[:],
                                        w_sb[:, hh * hq + ht, dsl],
                                        xt[:, ht, :],
                                        start=(hh == 0 and ht == 0),
                                        stop=(hh == HT // hq - 1
                                              and ht == hq - 1),
                                    )
                        for h in range(HEADS):
                            qst = a_st.tile([128, ST_A], F32R, tag="qst")
                            _rope_epilogue(
                                nc, a_st, ps_tiles[f"q{h}"], qst[:],
                                cos_sb[:, ssl], sin_sb[:, ssl], ST_A,
                            )
                            nc.gpsimd.dma_start(
                                out=qT_dram[h, :, :][:, ssl], in_=qst[:]
                            )
                        _rope_epilogue(
                            nc, a_st, ps_tiles["k"], kt_sb[:, ssl],
                            cos_sb[:, ssl], sin_sb[:, ssl], ST_A,
                        )
                        nc.vector.tensor_copy(vt_sb[:, ssl], ps_tiles["v"][:])
                        for kj in range(ST_A // 128):
                            ki = st * (ST_A // 128) + kj
                            tp = a_tps.tile([128, 128], F32, tag="tp",
                                            name=f"tp_{ki}")
                            nc.tensor.transpose(
                                tp[:], vt_sb[:, bass.ts(ki, 128)], ident_sb[:]
                            )
                            nc.vector.tensor_copy(v_sb[:, ki, :], tp[:])

            # ---- Phases B+C fused: per q-tile, attention for all 4 heads,
            # then immediately the output-projection matmuls for that q-tile.
            # C's independent matmuls fill B's softmax dependency bubbles.
            with tc.tile_pool(name="bc", bufs=1) as bc:
                attnT_sb = bc.tile([128, HEADS, S], F32R, tag="attnT")
                wo_sb = bc.tile([128, HEADS, H], F32R, tag="wo")

                with tc.tile_pool(name="b", bufs=1) as b, \
                     tc.tile_pool(name="b_p", bufs=6) as b_p, \
                     tc.tile_pool(name="b_da", bufs=3) as b_da, \
                     tc.tile_pool(name="b_r", bufs=2) as b_r, \
                     tc.tile_pool(name="c_st", bufs=4) as c_st, \
                     tc.tile_pool(name="b_sps", bufs=2, space="PSUM") as b_sps, \
                     tc.tile_pool(name="b_ops", bufs=2, space="PSUM") as b_ops, \
                     tc.tile_pool(name="b_aux", bufs=1, space="PSUM") as b_aux, \
                     tc.tile_pool(name="c_ps", bufs=3, space="PSUM") as c_ps:
                    masks_sb = b.tile([128, 4 * QT_W], F32R, tag="masks")
                    nc.scalar.dma_start(out=masks_sb[:], in_=masks[:])
                    for j in range(HEADS):
                        nc.scalar.dma_start(
                            out=wo_sb[:, j, :], in_=woT_t[:, j, :]
                        )

                    for qi in range(N_QT):
                        qsl = bass.ts(qi, QT_W)
                        n_k = 4 * qi + 4
                        for h in range(HEADS):
                            qt = b_qt.tile([128, QT_W], F32R, tag="qt")
                            nc.sync.dma_start(
                                out=qt[:], in_=qT_dram[h, :, :][:, qsl]
                            )
                            out_ps = b_ops.tile([128, QT_W], F32, tag="out")
                            den_acc = b_da.tile([128, QT_W], F32, tag="da")
                            for ki in range(n_k):
                                s_ps = b_sps.tile([128, QT_W], F32, tag="s")
                                nc.tensor.matmul(
                                    s_ps[:],
                                    (kt_sb[:, bass.ts(ki, 128)]),
                                    (qt[:]),
                                    start=True, stop=True,
                                )
                                p_t = b_p.tile([128, QT_W], F32R, tag="p")
                                nc.scalar.activation(
                                    p_t[:], s_ps[:],
                                    mybir.ActivationFunctionType.Exp,
                                    scale=SM_SCALE,
                                )
                                off = ki - 4 * qi
                                if off >= 0:
                                    nc.vector.tensor_mul(
                                        p_t[:], p_t[:],
                                        masks_sb[:, bass.ts(off, QT_W)],
                                    )
                                nc.tensor.matmul(
                                    out_ps[:], (v_sb[:, ki, :]), (p_t[:]),
                                    start=(ki == 0), stop=(ki == n_k - 1),
                                )
                                # denominator partials accumulate on DVE; one
                                # ones-matmul per q-tile reduces partitions
                                if ki == 0:
                                    nc.vector.tensor_copy(den_acc[:], p_t[:])
                                else:
                                    nc.vector.tensor_add(
                                        den_acc[:], den_acc[:], p_t[:]
                                    )
                            den_r = b_da.tile([128, QT_W], F32R, tag="dar")
                            nc.vector.tensor_copy(den_r[:], den_acc[:])
                            den_ps = b_aux.tile([128, QT_W], F32, tag="aux")
                            nc.tensor.matmul(
                                den_ps[0:1, :], (ones_sb[:, 0:1]), (den_r[:]),
                                start=True, stop=True,
                            )
                            recip = b_r.tile([1, QT_W], F32, tag="recip")
                            nc.vector.reciprocal(recip[:], den_ps[0:1, :])
                            recip_r = b_r.tile([1, QT_W], F32R, tag="recipr")
                            nc.vector.tensor_copy(recip_r[:], recip[:])
                            bc_ps = b_aux.tile([128, QT_W], F32, tag="aux")
                            nc.tensor.matmul(
                                bc_ps[:], (ones_sb[0:1, :]), (recip_r[:]),
                                start=True, stop=True,
                            )
                            bc_sb = b_r.tile([128, QT_W], F32, tag="bcs")
                            nc.scalar.copy(bc_sb[:], bc_ps[:])
                            nc.vector.tensor_mul(
                                attnT_sb[:, h, qsl], out_ps[:], bc_sb[:]
                            )

                        # output projection, pipelined one q-tile behind so
                        # these matmuls are dependency-free bubble fillers
                        cqis = [qi - 2] if qi >= 2 else []
                        if qi == N_QT - 1:
                            cqis += [qi - 1, qi]
                        for cqi in cqis:
                            for sj in range(QT_W // 128):
                                si = cqi * (QT_W // 128) + sj
                                for ei in range(H // ET):
                                    o_ps = c_ps.tile([128, ET], F32, tag="o")
                                    for j in range(HEADS):
                                        nc.tensor.matmul(
                                            o_ps[:],
                                            (attnT_sb[:, j, bass.ts(si, 128)]),
                                            (wo_sb[:, j, bass.ts(ei, ET)]),
                                            start=(j == 0),
                                            stop=(j == HEADS - 1),
                                        )
                                    o_st = c_st.tile([128, ET], F32, tag="ost")
                                    nc.scalar.copy(o_st[:], o_ps[:])
                                    nc.sync.dma_start(
                                        out=out[:][
                                            bass.ts(si, 128), bass.ts(ei, ET)
                                        ],
                                        in_=o_st[:],
                                    )
    _split_sync_waits(nc)
    return nc


_NC_CACHE = None


def _get_program():
    global _NC_CACHE
    if _NC_CACHE is None:
        _NC_CACHE = _build_program()
    return _NC_CACHE


def _host_tables(position_ids):
    pos = position_ids.reshape(-1).astype(np.float32)  # [S]
    inv_freq = (
        1.0
        / (np.float32(ROPE_THETA) ** (np.arange(0, D, 2, dtype=np.float32) / np.float32(D)))
    ).astype(np.float32)  # [64]
    freqs = pos[None, :] * inv_freq[:, None]  # [64, S]
    ang = np.concatenate([freqs, freqs], axis=0)  # [128, S]
    cosT = np.cos(ang).astype(np.float32)
    sinT = np.sin(ang).astype(np.float32)
    sinT[0:64, :] *= -1.0  # sign-fold for rotate_half

    masks = np.zeros((128, 4 * QT_W), dtype=np.float32)
    for off in range(4):
        p = np.arange(128)[:, None]
        c = np.arange(QT_W)[None, :]
        masks[:, off * QT_W : (off + 1) * QT_W] = (128 * off + p <= c).astype(
            np.float32
        )
    return cosT, sinT, masks


def _prepare_in_maps(hidden_states, Wq, Wk, Wv, Wo, position_ids):
    x = np.asarray(hidden_states, dtype=np.float32).reshape(S, H)
    Wq = np.asarray(Wq, dtype=np.float32)
    Wk = np.asarray(Wk, dtype=np.float32)
    Wv = np.asarray(Wv, dtype=np.float32)
    Wo = np.asarray(Wo, dtype=np.float32)

    xT = np.ascontiguousarray(x.T)  # [H, S]
    cosT, sinT, masks = _host_tables(np.asarray(position_ids))
    ident = np.eye(128, dtype=np.float32)
    ones = np.ones((128, 128), dtype=np.float32)

    in_maps = []
    for c in range(N_CORES):
        qs = slice(DQ * c, DQ * (c + 1))
        ks = slice(DKV * c, DKV * (c + 1))
        in_maps.append(
            {
                "xT": xT,
                "wqT": np.ascontiguousarray(Wq[qs, :].T),
                "wkT": np.ascontiguousarray(Wk[ks, :].T),
                "wvT": np.ascontiguousarray(Wv[ks, :].T),
                "woT": np.ascontiguousarray(Wo[:, qs].T),
                "cosT": cosT,
                "sinT": sinT,
                "masks": masks,
                "ident": ident,
                "ones": ones,
            }
        )
    return in_maps


def _finalize(results, batch):
    out = np.zeros((S, H), dtype=np.float32)
    for c in range(N_CORES):
        out += results[c]["out"]
    return out.reshape(batch, S, H)


def kernel(hidden_states, Wq, Wk, Wv, Wo, position_ids):
    from concourse.bass_utils import run_bass_kernel_spmd

    B = hidden_states.shape[0]
    in_maps = _prepare_in_maps(hidden_states, Wq, Wk, Wv, Wo, position_ids)
    nc = _get_program()
    res = run_bass_kernel_spmd(nc, in_maps, list(range(N_CORES)))
    return _finalize(res.results, B)



# revision 2
# speedup vs baseline: 1.1219x; 1.1219x over previous
"""Tensor-parallel LlamaAttention v2 — bf16 edition.

Same sharding as v1 (core c owns q-heads 4c..4c+3, kv head c), but:
  - every matmul operand is bf16 (PE rate unchanged vs f32r, DMA/SBUF halved)
  - qT stays SBUF-resident (no DRAM round-trip between phases)
  - scores are computed in ki-PAIRS ([128, 2, 512] PSUM tiles) so one ACT
    exp instruction covers 1024 columns
  - phase B interleaves head pairs, and phase C's output-projection matmuls
    for q-tile qi-1 are interspersed between B(qi)'s steps via generators
  - output partials are written bf16 and summed on host in fp32
"""

import math
import sys

sys.path.insert(0, "/opt/trn_rl_repo")

import numpy as np

import concourse.bass as bass
import concourse.mybir as mybir
import concourse.tile as tile_mod
from concourse.tile import ScopedClock

F32 = mybir.dt.float32
BF16 = mybir.dt.bfloat16

S = 2048
H = 4096
DQ = 512  # per-core query width (4 heads x 128)
DKV = 128  # per-core kv width (1 head)
D = 128  # head dim
N_CORES = 8
HEADS = 4  # q heads per core
ROPE_THETA = 500000.0
SM_SCALE = 1.0 / math.sqrt(D)

HT = H // 128  # 32 contraction tiles
ST_A = 512  # pass-A moving-operand width
N_ST_A = S // ST_A
HQ = 8  # h-tiles per X chunk
QT_W = 512  # phase-B q-tile width
N_QT = S // QT_W
N_KT = S // 128  # 16 k-tiles of 128
ET = 512  # phase-C output e-tile width


def _patch_tilecontext():
    """walrus's CTRL codegen rejects >2 sync waits on one instruction; the
    Tile kernel-tail drain waits on the whole global clock. Spread the waits
    one-per-nop before the drain."""
    if getattr(tile_mod.TileContext, "_drain_patched", False):
        return

    def _drain_and_barrier(self, tick_clock, wait_clock):
        nc = self.nc
        probe = nc.sync.nop(nofuse=True)
        wait_clock.add_sem_waits(
            probe.ins, ScopedClock({None: tick_clock.global_clock})
        )
        si = probe.ins.sync_info
        waits = list(si.on_wait or [])
        if len(waits) > 1:
            si.on_wait = waits[:1]
            for w in waits[1:]:
                n = nc.sync.nop(nofuse=True)
                if n.ins.sync_info is None:
                    n.ins.sync_info = mybir.SyncInfo(on_wait=[w], on_update=[])
                else:
                    n.ins.sync_info.on_wait = [w]
        nc.sync.drain()
        nc.all_engine_barrier()
        assert self.sems is not None
        popped = nc._tile_sem_poison_stack.pop()
        assert popped is self._sem_poison
        nc.clear_and_free_semaphores(list(self.sems.allocated().values()))
        nc.all_engine_barrier()

    tile_mod.TileContext._drain_and_barrier = _drain_and_barrier
    tile_mod.TileContext._drain_patched = True


def _split_sync_waits(nc, cap=1):
    """walrus's CoreV3 codegen rejects instructions carrying more than ~2
    sync-wait commands. Hoist extra waits onto nops inserted just before the
    instruction on the same engine."""
    n_split = 0
    for fn in nc.m.functions:
        for bb in fn.blocks:
            new_insts = []
            for inst in bb.instructions:
                si = inst.sync_info
                waits = list(si.on_wait) if si and si.on_wait else []
                if len(waits) > cap:
                    keep = waits[-cap:]
                    for j, w in enumerate(waits[:-cap]):
                        nop = mybir.InstNoOp(
                            name=f"{inst.name}-wsplit{j}", ins=[], outs=[]
                        )
                        nop.engine = inst.engine
                        nop.sync_info = mybir.SyncInfo(on_wait=[w], on_update=[])
                        new_insts.append(nop)
                        n_split += 1
                    si.on_wait = keep
                new_insts.append(inst)
            bb.instructions[:] = new_insts
    return n_split


def _rope_epilogue(nc, pool, ps, out_ap, cos_ap, sin_ap, width):
    """out(bf16) = ps * cos + rotate_half(ps) * sin_signed, out of PSUM.

    sin_ap carries the sign fold: rows 0:64 hold -sin, rows 64:128 hold +sin,
    so rotate_half is just a 64-partition swap on the ps read."""
    _rope_epilogue.n += 1
    t1 = pool.tile([128, ST_A], BF16, tag="rope_t1",
                   name=f"rt1_{_rope_epilogue.n}")[:, 0:width]
    t2 = pool.tile([128, ST_A], BF16, tag="rope_t2",
                   name=f"rt2_{_rope_epilogue.n}")[:, 0:width]
    nc.vector.tensor_mul(t1, ps[:], cos_ap)
    nc.vector.tensor_mul(t2[0:64, :], ps[64:128, :], sin_ap[0:64, :])
    nc.vector.tensor_mul(t2[64:128, :], ps[0:64, :], sin_ap[64:128, :])
    nc.vector.tensor_add(out_ap, t1, t2)


_rope_epilogue.n = 0


def _build_program():
    _patch_tilecontext()
    nc = bass.Bass()

    xT = nc.declare_dram_parameter("xT", [H, S], BF16, isOutput=False)
    wqT = nc.declare_dram_parameter("wqT", [H, DQ], BF16, isOutput=False)
    wkT = nc.declare_dram_parameter("wkT", [H, DKV], BF16, isOutput=False)
    wvT = nc.declare_dram_parameter("wvT", [H, DKV], BF16, isOutput=False)
    woT = nc.declare_dram_parameter("woT", [DQ, H], BF16, isOutput=False)
    cosT = nc.declare_dram_parameter("cosT", [D, S], BF16, isOutput=False)
    sinT = nc.declare_dram_parameter("sinT", [D, S], BF16, isOutput=False)
    masks = nc.declare_dram_parameter("masks", [128, 4 * QT_W], BF16, isOutput=False)
    ident = nc.declare_dram_parameter("ident", [128, 128], BF16, isOutput=False)
    ones = nc.declare_dram_parameter("ones", [128, 128], BF16, isOutput=False)
    out = nc.declare_dram_parameter("out", [S, H], BF16, isOutput=True)

    xT_t = xT[:].rearrange("(ht p) s -> p ht s", p=128)
    wqT_t = wqT[:].rearrange("(ht p) d -> p ht d", p=128)
    wkT_t = wkT[:].rearrange("(ht p) d -> p ht d", p=128)
    wvT_t = wvT[:].rearrange("(ht p) d -> p ht d", p=128)
    woT_t = woT[:].rearrange("(j p) e -> p j e", p=128)

    from contextlib import ExitStack

    with nc.allow_low_precision("bf16 attention kernel by design"), \
         tile_mod.TileContext(nc) as tc:
        with ExitStack() as _stk:
            # whole-kernel residents
            persist = _stk.enter_context(tc.tile_pool(name="persist", bufs=1))
            qT_sb = persist.tile([128, HEADS, S], BF16)  # roped Q, d x s per head
            kt_sb = persist.tile([128, S], BF16)  # roped K.T (d x k)
            v_sb = persist.tile([128, N_KT, 128], BF16)  # V natural (k x d)
            ones_sb = persist.tile([128, 128], BF16)

            # early phase-B assets in disjoint space so their DMAs can land
            # during phase A without alias-deps on A pools
            bc_pool = _stk.enter_context(tc.tile_pool(name="bc", bufs=1))
            wo_sb = bc_pool.tile([128, HEADS, H], BF16, tag="wo")
            masks_sb = bc_pool.tile([128, 4, QT_W], BF16, tag="masks")

            # ---- Pass A: QKV projections in one X sweep; 6 PSUM banks
            # accumulate k/v/q0..q3 over H per s-tile.
            with tc.tile_pool(name="aw", bufs=1) as aw, \
                 tc.tile_pool(name="a_xt", bufs=4) as a_xt, \
                 tc.tile_pool(name="a_st", bufs=3) as a_st, \
                 tc.tile_pool(name="cs", bufs=1) as cs, \
                 tc.tile_pool(name="a_ps", bufs=1, space="PSUM") as a_ps, \
                 tc.tile_pool(name="a_tps", bufs=2, space="PSUM") as a_tps:
                cos_sb = cs.tile([128, S], BF16, tag="cos")
                sin_sb = cs.tile([128, S], BF16, tag="sin")

                # Single DMA queue (sync), strict consumption order: the cost
                # model serializes all transfers through one DMA device, so
                # global transfer order must match PE consumption order.
                wk_sb = aw.tile([128, HT, DKV], BF16, tag="wk")
                wv_sb = aw.tile([128, HT, DKV], BF16, tag="wv")
                wq_sb = aw.tile([128, HT, DQ], BF16, tag="wq")
                ident_sb = aw.tile([128, 128], BF16, tag="ident")
                vt_sb = aw.tile([128, S], BF16, tag="vt")  # V.T staging

                def load_w(hsl):
                    nc.sync.dma_start(out=wk_sb[:, hsl, :], in_=wkT_t[:, hsl, :])
                    nc.sync.dma_start(out=wv_sb[:, hsl, :], in_=wvT_t[:, hsl, :])
                    nc.sync.dma_start(out=wq_sb[:, hsl, :], in_=wqT_t[:, hsl, :])

                dlist = [(wk_sb, slice(0, D), "k"), (wv_sb, slice(0, D), "v")]
                dlist += [(wq_sb, bass.ts(h, D), f"q{h}") for h in range(HEADS)]

                s_tiles = [(ST_A * i, ST_A) for i in range(N_ST_A)]
                n_s = len(s_tiles)

                xt00 = a_xt.tile([128, 2, ST_A], BF16, tag="xt0", name="xt00")
                nc.sync.dma_start(out=xt00[:], in_=xT_t[:, 0:2, 0:ST_A])
                load_w(slice(0, 2))
                xt01 = a_xt.tile([128, 2, ST_A], BF16, tag="xt0", name="xt01")
                nc.sync.dma_start(out=xt01[:], in_=xT_t[:, 2:4, 0:ST_A])

                for st, (s_off, s_w) in enumerate(s_tiles):
                    if st == n_s - 1:
                        # phase-B assets: on the sync queue between the last
                        # two s-tiles' X chunks so queue order delays their
                        # transfers past the earlier X streaming
                        nc.sync.dma_start(
                            out=masks_sb[:],
                            in_=masks[:].rearrange("p (o w) -> p o w", w=QT_W),
                        )
                        nc.sync.dma_start(out=ones_sb[:], in_=ones[:])
                        for j in range(HEADS):
                            nc.sync.dma_start(
                                out=wo_sb[:, j, :], in_=woT_t[:, j, :]
                            )
                    ssl = slice(s_off, s_off + s_w)
                    ps_tiles = {}
                    for _, _, kind in dlist:
                        ps_tiles[kind] = a_ps.tile(
                            [128, ST_A], F32, tag=f"mm_{kind}",
                            name=f"ps_{kind}_{st}",
                        )
                    if st == 0:
                        chunks = [(xt00, 0, 2), (xt01, 2, 4), (None, 4, 8),
                                  (None, 8, 12), (None, 12, 16),
                                  (None, 16, 24), (None, 24, 32)]
                    else:
                        chunks = [(None, c, c + HQ) for c in range(0, HT, HQ)]
                    xts = []
                    for ci, (xt_pre, h0, h1) in enumerate(chunks):
                        if xt_pre is not None:
                            xt = xt_pre
                        else:
                            xt = a_xt.tile([128, h1 - h0, s_w], BF16,
                                           tag="xt", name=f"xt_{st}_{ci}")
                            nc.sync.dma_start(
                                out=xt[:], in_=xT_t[:, h0:h1, ssl],
                            )
                        xts.append((xt, h0, h1))
                        if st == 0:
                            # weight chunks interleave in consumption order
                            w_sched = {2: (2, 4), 4: (4, 8), 8: (8, 12),
                                       12: (12, 16), 16: (16, 24),
                                       24: (24, 32)}
                            if h1 in w_sched:
                                load_w(slice(*w_sched[h1]))
                            elif h1 == HT:
                                # in time for st=0's rope epilogue
                                nc.sync.dma_start(out=cos_sb[:], in_=cosT[:])
                                nc.sync.dma_start(out=sin_sb[:], in_=sinT[:])
                                nc.sync.dma_start(out=ident_sb[:], in_=ident[:])
                        if st < n_s - 1:
                            for w_sb, dsl, kind in dlist:
                                ps = ps_tiles[kind]
                                for ht in range(h1 - h0):
                                    nc.tensor.matmul(
                                        ps[:, 0:s_w],
                                        w_sb[:, h0 + ht, dsl],
                                        xt[:, ht, :],
                                        start=(h0 == 0 and ht == 0),
                                        stop=(h1 == HT and ht == h1 - h0 - 1),
                                    )
                    if st == n_s - 1:
                        # Last s-tile: one output at a time, with its rope
                        # epilogue emitted immediately — each epilogue overlaps
                        # the next output's matmuls, so phase B never waits on
                        # a serial rope tail (and the k/v PSUM banks that B's
                        # score tiles will alias free ~27us before B starts).
                        for w_sb, dsl, kind in dlist:
                            ps = ps_tiles[kind]
                            for xt, h0, h1 in xts:
                                for ht in range(h1 - h0):
                                    nc.tensor.matmul(
                                        ps[:, 0:s_w],
                                        w_sb[:, h0 + ht, dsl],
                                        xt[:, ht, :],
                                        start=(h0 == 0 and ht == 0),
                                        stop=(h1 == HT and ht == h1 - h0 - 1),
                                    )
                            if kind == "k":
                                _rope_epilogue(
                                    nc, a_st, ps[:, 0:s_w], kt_sb[:, ssl],
                                    cos_sb[:, ssl], sin_sb[:, ssl], s_w,
                                )
                            elif kind == "v":
                                nc.vector.tensor_copy(vt_sb[:, ssl],
                                                      ps[:, 0:s_w])
                            else:
                                h = int(kind[1])
                                _rope_epilogue(
                                    nc, a_st, ps[:, 0:s_w], qT_sb[:, h, ssl],
                                    cos_sb[:, ssl], sin_sb[:, ssl], s_w,
                                )
                        # transposes last: their vt dependency cleared long
                        # ago, so PE never stalls on the DVE copy
                        for kj in range(s_w // 128):
                            ki = s_off // 128 + kj
                            tp = a_tps.tile([128, 128], BF16, tag="tp",
                                            name=f"tp_{ki}")
                            nc.tensor.transpose(
                                tp[:], vt_sb[:, bass.ts(ki, 128)], ident_sb[:]
                            )
                            nc.vector.tensor_copy(v_sb[:, ki, :], tp[:])
                        continue
                    # k/v epilogues first: frees their PSUM banks earliest
                    _rope_epilogue(
                        nc, a_st, ps_tiles["k"][:, 0:s_w], kt_sb[:, ssl],
                        cos_sb[:, ssl], sin_sb[:, ssl], s_w,
                    )
                    nc.vector.tensor_copy(vt_sb[:, ssl],
                                          ps_tiles["v"][:, 0:s_w])
                    for h in range(HEADS):
                        _rope_epilogue(
                            nc, a_st, ps_tiles[f"q{h}"][:, 0:s_w],
                            qT_sb[:, h, ssl],
                            cos_sb[:, ssl], sin_sb[:, ssl], s_w,
                        )
                    for kj in range(s_w // 128):
                        ki = s_off // 128 + kj
                        tp = a_tps.tile([128, 128], BF16, tag="tp",
                                        name=f"tp_{ki}")
                        nc.tensor.transpose(
                            tp[:], vt_sb[:, bass.ts(ki, 128)], ident_sb[:]
                        )
                        nc.vector.tensor_copy(v_sb[:, ki, :], tp[:])

            # ---- Phases B+C fused via generators.
            with tc.tile_pool(name="b_p", bufs=6) as b_p, \
                 tc.tile_pool(name="b_da", bufs=3) as b_da, \
                 tc.tile_pool(name="b_r", bufs=2) as b_r, \
                 tc.tile_pool(name="c_row", bufs=2) as c_row, \
                 tc.tile_pool(name="attn", bufs=1) as attn_pool, \
                 tc.tile_pool(name="b_sps", bufs=2, space="PSUM") as b_sps, \
                 tc.tile_pool(name="b_ops", bufs=2, space="PSUM") as b_ops, \
                 tc.tile_pool(name="cd_ps", bufs=2, space="PSUM") as cd_ps:
                attnT_sb = attn_pool.tile([128, HEADS, S], BF16, tag="attnT")

                def b_steps(qi):
                    """Emit phase-B work for q-tile qi; yields at natural
                    C-interleave points."""
                    qsl = bass.ts(qi, QT_W)
                    n_pairs = 2 * qi + 2
                    for hp in range(2):
                        hs = (2 * hp, 2 * hp + 1)
                        ops = {}
                        dens = {}
                        for h in hs:
                            ops[h] = b_ops.tile([128, QT_W], F32, tag="out",
                                                name=f"op_{qi}_{h}")
                            dens[h] = b_da.tile([128, QT_W], BF16, tag="da",
                                                name=f"da_{qi}_{h}")
                        for pk in range(n_pairs):
                            p_tiles = {}
                            for h in hs:
                                qt_ap = qT_sb[:, h, qsl]
                                sps = b_sps.tile([128, 2, QT_W], F32, tag="s",
                                                 name=f"s_{qi}_{h}_{pk}")
                                for j in range(2):
                                    ki = 2 * pk + j
                                    nc.tensor.matmul(
                                        sps[:, j, :],
                                        kt_sb[:, bass.ts(ki, 128)],
                                        qt_ap,
                                        start=True, stop=True,
                                    )
                                p_t = b_p.tile([128, 2, QT_W], BF16, tag="p",
                                               name=f"p_{qi}_{h}_{pk}")
                                p_tiles[h] = p_t
                                nc.scalar.activation(
                                    p_t[:], sps[:],
                                    mybir.ActivationFunctionType.Exp,
                                    scale=SM_SCALE,
                                )
                                if pk >= n_pairs - 2:
                                    off = 2 * (pk - (n_pairs - 2))
                                    nc.vector.tensor_mul(
                                        p_t[:], p_t[:],
                                        masks_sb[:, off : off + 2, :],
                                    )
                            yield  # C fills the exp->AV latency here
                            for h in hs:
                                p_t = p_tiles[h]
                                for j in range(2):
                                    ki = 2 * pk + j
                                    nc.tensor.matmul(
                                        ops[h][:], v_sb[:, ki, :], p_t[:, j, :],
                                        start=(pk == 0 and j == 0),
                                        stop=(pk == n_pairs - 1 and j == 1),
                                    )
                                if pk == 0:
                                    nc.vector.tensor_add(
                                        dens[h][:], p_t[:, 0, :], p_t[:, 1, :]
                                    )
                                else:
                                    nc.vector.tensor_add(
                                        dens[h][:], dens[h][:], p_t[:, 0, :]
                                    )
                                    nc.vector.tensor_add(
                                        dens[h][:], dens[h][:], p_t[:, 1, :]
                                    )
                            yield
                        for h in hs:
                            den_ps = cd_ps.tile([128, QT_W], F32, tag="cd",
                                                name=f"den_{qi}_{h}")
                            nc.tensor.matmul(
                                den_ps[0:1, :], ones_sb[:, 0:1], dens[h][:],
                                start=True, stop=True,
                            )
                            recip = b_r.tile([1, QT_W], BF16, tag="recip")
                            nc.vector.reciprocal(recip[:], den_ps[0:1, :])
                            yield  # C fills the recip->bcast latency
                            bc_ps = cd_ps.tile([128, QT_W], F32, tag="cd",
                                               name=f"bc_{qi}_{h}")
                            nc.tensor.matmul(
                                bc_ps[:], ones_sb[0:1, :], recip[:],
                                start=True, stop=True,
                            )
                            bc_sb = b_r.tile([128, QT_W], F32, tag="bcs")
                            nc.scalar.copy(bc_sb[:], bc_ps[:])
                            nc.vector.tensor_mul(
                                attnT_sb[:, h, qsl], ops[h][:], bc_sb[:]
                            )
                            yield

                def c_steps(cqi):
                    """Output projection for q-tile cqi: 4 si x 8 ei groups,
                    staged into [128, H] rows. PSUM->SBUF copies alternate
                    ACT/DVE to keep ACT headroom for the exp chain."""
                    last_si = cqi == N_QT - 1
                    for sj in range(QT_W // 128):
                        si = cqi * (QT_W // 128) + sj
                        tail = last_si and sj == QT_W // 128 - 1
                        o_row = c_row.tile([128, H // ET, ET], BF16, tag="orow",
                                           name=f"orow_{si}")
                        n_e = H // ET
                        for ei in range(n_e):
                            o_ps = cd_ps.tile([128, ET], F32, tag="cd",
                                              name=f"o_{si}_{ei}")
                            for j in range(HEADS):
                                nc.tensor.matmul(
                                    o_ps[:],
                                    attnT_sb[:, j, bass.ts(si, 128)],
                                    wo_sb[:, j, bass.ts(ei, ET)],
                                    start=(j == 0),
                                    stop=(j == HEADS - 1),
                                )
                            if ei % 2 == 0:
                                nc.scalar.copy(o_row[:, ei, :], o_ps[:])
                            else:
                                nc.vector.tensor_copy(o_row[:, ei, :], o_ps[:])
                            # stream the row out: on the tail group, one DMA
                            # per copy spread over queues (shortest drain);
                            # halves otherwise
                            step = 1 if tail else 4
                            if (ei + 1) % step == 0:
                                e0 = ei + 1 - step
                                eng = (
                                    [nc.gpsimd, nc.sync][ei % 2]
                                    if tail else nc.gpsimd
                                )
                                eng.dma_start(
                                    out=out[:][
                                        bass.ts(si, 128), e0 * ET : (ei + 1) * ET
                                    ],
                                    in_=o_row[:, e0 : ei + 1, :].rearrange(
                                        "p e w -> p (e w)"
                                    ),
                                )
                            yield

                def drive(bgen, cgen, rate):
                    credit = 0.0
                    for _ in bgen:
                        credit += rate
                        while credit >= 1.0:
                            credit -= 1.0
                            if next(cgen, None) is None:
                                credit = 0.0
                                break

                pending_c = iter(())
                for qi in range(N_QT):
                    # b yields: 2 * (2 * n_pairs + 2); c steps pending: 32
                    n_b = 2 * (2 * (2 * qi + 2) + 2)
                    rate = 32.0 / n_b if qi > 0 else 0.0
                    drive(b_steps(qi), pending_c, rate)
                    # drain any leftover C steps for qi-1
                    for _ in pending_c:
                        pass
                    pending_c = c_steps(qi)
                # tail: C for the last q-tile
                for _ in pending_c:
                    pass
    _split_sync_waits(nc)
    return nc


_NC_CACHE = None


def _get_program():
    global _NC_CACHE
    if _NC_CACHE is None:
        _NC_CACHE = _build_program()
    return _NC_CACHE


def _host_tables(position_ids):
    pos = position_ids.reshape(-1).astype(np.float32)  # [S]
    inv_freq = (
        1.0
        / (np.float32(ROPE_THETA) ** (np.arange(0, D, 2, dtype=np.float32) / np.float32(D)))
    ).astype(np.float32)  # [64]
    freqs = pos[None, :] * inv_freq[:, None]  # [64, S]
    ang = np.concatenate([freqs, freqs], axis=0)  # [128, S]
    cosT = np.cos(ang).astype(np.float32)
    sinT = np.sin(ang).astype(np.float32)
    sinT[0:64, :] *= -1.0  # sign-fold for rotate_half

    masks = np.zeros((128, 4 * QT_W), dtype=np.float32)
    for off in range(4):
        p = np.arange(128)[:, None]
        c = np.arange(QT_W)[None, :]
        masks[:, off * QT_W : (off + 1) * QT_W] = (128 * off + p <= c).astype(
            np.float32
        )
    return cosT, sinT, masks


def _prepare_in_maps(hidden_states, Wq, Wk, Wv, Wo, position_ids):
    bf16 = mybir.dt.np(mybir.dt.bfloat16)
    x = np.asarray(hidden_states, dtype=np.float32).reshape(S, H)
    Wq = np.asarray(Wq, dtype=np.float32)
    Wk = np.asarray(Wk, dtype=np.float32)
    Wv = np.asarray(Wv, dtype=np.float32)
    Wo = np.asarray(Wo, dtype=np.float32)

    xT = np.ascontiguousarray(x.T).astype(bf16)  # [H, S]
    cosT, sinT, masks = _host_tables(np.asarray(position_ids))
    cosT = cosT.astype(bf16)
    sinT = sinT.astype(bf16)
    masks = masks.astype(bf16)
    ident = np.eye(128, dtype=np.float32).astype(bf16)
    ones = np.ones((128, 128), dtype=np.float32).astype(bf16)

    in_maps = []
    for c in range(N_CORES):
        qs = slice(DQ * c, DQ * (c + 1))
        ks = slice(DKV * c, DKV * (c + 1))
        in_maps.append(
            {
                "xT": xT,
                "wqT": np.ascontiguousarray(Wq[qs, :].T).astype(bf16),
                "wkT": np.ascontiguousarray(Wk[ks, :].T).astype(bf16),
                "wvT": np.ascontiguousarray(Wv[ks, :].T).astype(bf16),
                "woT": np.ascontiguousarray(Wo[:, qs].T).astype(bf16),
                "cosT": cosT,
                "sinT": sinT,
                "masks": masks,
                "ident": ident,
                "ones": ones,
            }
        )
    return in_maps


def _finalize(results, batch):
    out = np.zeros((S, H), dtype=np.float32)
    for c in range(N_CORES):
        out += results[c]["out"].astype(np.float32)
    return out.reshape(batch, S, H)


def kernel(hidden_states, Wq, Wk, Wv, Wo, position_ids):
    from concourse.bass_utils import run_bass_kernel_spmd

    B = hidden_states.shape[0]
    in_maps = _prepare_in_maps(hidden_states, Wq, Wk, Wv, Wo, position_ids)
    nc = _get_program()
    res = run_bass_kernel_spmd(nc, in_maps, list(range(N_CORES)))
    return _finalize(res.results, B)


# revision 3
# speedup vs baseline: 1.4791x; 1.3184x over previous
"""Tensor-parallel LlamaAttention v2 — bf16 edition.

Same sharding as v1 (core c owns q-heads 4c..4c+3, kv head c), but:
  - every matmul operand is bf16 (PE rate unchanged vs f32r, DMA/SBUF halved)
  - qT stays SBUF-resident (no DRAM round-trip between phases)
  - scores are computed in ki-PAIRS ([128, 2, 512] PSUM tiles) so one ACT
    exp instruction covers 1024 columns
  - phase B interleaves head pairs, and phase C's output-projection matmuls
    for q-tile qi-1 are interspersed between B(qi)'s steps via generators
  - output partials are written bf16 and summed on host in fp32
"""

import math
import sys

sys.path.insert(0, "/opt/trn_rl_repo")

import numpy as np

import concourse.bass as bass
import concourse.mybir as mybir
import concourse.tile as tile_mod
from concourse.tile import ScopedClock

F32 = mybir.dt.float32
BF16 = mybir.dt.bfloat16

S = 2048
H = 4096
DQ = 512  # per-core query width (4 heads x 128)
DKV = 128  # per-core kv width (1 head)
D = 128  # head dim
N_CORES = 8
HEADS = 4  # q heads per core
ROPE_THETA = 500000.0
SM_SCALE = 1.0 / math.sqrt(D)

HT = H // 128  # 32 contraction tiles
ST_A = 512  # pass-A moving-operand width
N_ST_A = S // ST_A
HQ = 8  # h-tiles per X chunk
QT_W = 512  # phase-B q-tile width
N_QT = S // QT_W
N_KT = S // 128  # 16 k-tiles of 128
ET = 512  # phase-C output e-tile width


def _patch_tilecontext():
    """walrus's CTRL codegen rejects >2 sync waits on one instruction; the
    Tile kernel-tail drain waits on the whole global clock. Spread the waits
    one-per-nop before the drain."""
    if getattr(tile_mod.TileContext, "_drain_patched", False):
        return

    def _drain_and_barrier(self, tick_clock, wait_clock):
        nc = self.nc
        probe = nc.sync.nop(nofuse=True)
        wait_clock.add_sem_waits(
            probe.ins, ScopedClock({None: tick_clock.global_clock})
        )
        si = probe.ins.sync_info
        waits = list(si.on_wait or [])
        if len(waits) > 1:
            si.on_wait = waits[:1]
            for w in waits[1:]:
                n = nc.sync.nop(nofuse=True)
                if n.ins.sync_info is None:
                    n.ins.sync_info = mybir.SyncInfo(on_wait=[w], on_update=[])
                else:
                    n.ins.sync_info.on_wait = [w]
        nc.sync.drain()
        nc.all_engine_barrier()
        assert self.sems is not None
        popped = nc._tile_sem_poison_stack.pop()
        assert popped is self._sem_poison
        nc.clear_and_free_semaphores(list(self.sems.allocated().values()))
        nc.all_engine_barrier()

    tile_mod.TileContext._drain_and_barrier = _drain_and_barrier
    tile_mod.TileContext._drain_patched = True


def _split_sync_waits(nc, cap=1):
    """walrus's CoreV3 codegen rejects instructions carrying more than ~2
    sync-wait commands. Hoist extra waits onto nops inserted just before the
    instruction on the same engine."""
    n_split = 0
    for fn in nc.m.functions:
        for bb in fn.blocks:
            new_insts = []
            for inst in bb.instructions:
                si = inst.sync_info
                waits = list(si.on_wait) if si and si.on_wait else []
                if len(waits) > cap:
                    keep = waits[-cap:]
                    for j, w in enumerate(waits[:-cap]):
                        nop = mybir.InstNoOp(
                            name=f"{inst.name}-wsplit{j}", ins=[], outs=[]
                        )
                        nop.engine = inst.engine
                        nop.sync_info = mybir.SyncInfo(on_wait=[w], on_update=[])
                        new_insts.append(nop)
                        n_split += 1
                    si.on_wait = keep
                new_insts.append(inst)
            bb.instructions[:] = new_insts
    return n_split


def _rope_epilogue(nc, pool, ps, out_ap, cos_ap, sin_ap, width):
    """out(bf16) = ps * cos + rotate_half(ps) * sin_signed, out of PSUM.

    sin_ap carries the sign fold: rows 0:64 hold -sin, rows 64:128 hold +sin,
    so rotate_half is just a 64-partition swap on the ps read."""
    _rope_epilogue.n += 1
    t1 = pool.tile([128, ST_A], BF16, tag="rope_t1",
                   name=f"rt1_{_rope_epilogue.n}")[:, 0:width]
    t2 = pool.tile([128, ST_A], BF16, tag="rope_t2",
                   name=f"rt2_{_rope_epilogue.n}")[:, 0:width]
    nc.vector.tensor_mul(t1, ps[:], cos_ap)
    nc.vector.tensor_mul(t2[0:64, :], ps[64:128, :], sin_ap[0:64, :])
    nc.vector.tensor_mul(t2[64:128, :], ps[0:64, :], sin_ap[64:128, :])
    nc.vector.tensor_add(out_ap, t1, t2)


_rope_epilogue.n = 0


def _build_program():
    _patch_tilecontext()
    nc = bass.Bass()

    xT = nc.declare_dram_parameter("xT", [H, S], BF16, isOutput=False)
    wqT = nc.declare_dram_parameter("wqT", [H, DQ], BF16, isOutput=False)
    wkT = nc.declare_dram_parameter("wkT", [H, DKV], BF16, isOutput=False)
    wvT = nc.declare_dram_parameter("wvT", [H, DKV], BF16, isOutput=False)
    woT = nc.declare_dram_parameter("woT", [DQ, H], BF16, isOutput=False)
    cosT = nc.declare_dram_parameter("cosT", [D, S], BF16, isOutput=False)
    sinT = nc.declare_dram_parameter("sinT", [D, S], BF16, isOutput=False)
    masks = nc.declare_dram_parameter("masks", [128, 4 * QT_W], BF16, isOutput=False)
    ident = nc.declare_dram_parameter("ident", [128, 128], BF16, isOutput=False)
    ones = nc.declare_dram_parameter("ones", [128, 128], BF16, isOutput=False)
    out = nc.declare_dram_parameter("out", [S, H], BF16, isOutput=True)

    xT_t = xT[:].rearrange("(ht p) s -> p ht s", p=128)
    wqT_t = wqT[:].rearrange("(ht p) d -> p ht d", p=128)
    wkT_t = wkT[:].rearrange("(ht p) d -> p ht d", p=128)
    wvT_t = wvT[:].rearrange("(ht p) d -> p ht d", p=128)
    woT_t = woT[:].rearrange("(j p) e -> p j e", p=128)

    from contextlib import ExitStack

    with nc.allow_low_precision("bf16 attention kernel by design"), \
         tile_mod.TileContext(nc) as tc:
        with ExitStack() as _stk:
            # whole-kernel residents
            persist = _stk.enter_context(tc.tile_pool(name="persist", bufs=1))
            qT_sb = persist.tile([128, HEADS, S], BF16)  # roped Q, d x s per head
            kt_sb = persist.tile([128, S], BF16)  # roped K.T (d x k)
            v_sb = persist.tile([128, N_KT, 128], BF16)  # V natural (k x d)
            ones_sb = persist.tile([128, 128], BF16)

            # early phase-B assets in disjoint space so their DMAs can land
            # during phase A without alias-deps on A pools
            bc_pool = _stk.enter_context(tc.tile_pool(name="bc", bufs=1))
            wo_sb = bc_pool.tile([128, HEADS, H], BF16, tag="wo")
            masks_sb = bc_pool.tile([128, 4, QT_W], BF16, tag="masks")

            # ---- Pass A: QKV projections in one X sweep; 6 PSUM banks
            # accumulate k/v/q0..q3 over H per s-tile.
            with tc.tile_pool(name="aw", bufs=1) as aw, \
                 tc.tile_pool(name="a_xt", bufs=4) as a_xt, \
                 tc.tile_pool(name="a_st", bufs=3) as a_st, \
                 tc.tile_pool(name="cs", bufs=1) as cs, \
                 tc.tile_pool(name="a_ps", bufs=1, space="PSUM") as a_ps, \
                 tc.tile_pool(name="a_tps", bufs=2, space="PSUM") as a_tps:
                cos_sb = cs.tile([128, S], BF16, tag="cos")
                sin_sb = cs.tile([128, S], BF16, tag="sin")

                # Single DMA queue (sync), strict consumption order: the cost
                # model serializes all transfers through one DMA device, so
                # global transfer order must match PE consumption order.
                wk_sb = aw.tile([128, HT, DKV], BF16, tag="wk")
                wv_sb = aw.tile([128, HT, DKV], BF16, tag="wv")
                wq_sb = aw.tile([128, HT, DQ], BF16, tag="wq")
                ident_sb = aw.tile([128, 128], BF16, tag="ident")
                vt_sb = aw.tile([128, S], BF16, tag="vt")  # V.T staging

                def load_w(hsl):
                    nc.sync.dma_start(out=wk_sb[:, hsl, :], in_=wkT_t[:, hsl, :])
                    nc.sync.dma_start(out=wv_sb[:, hsl, :], in_=wvT_t[:, hsl, :])
                    nc.sync.dma_start(out=wq_sb[:, hsl, :], in_=wqT_t[:, hsl, :])

                dlist = [(wk_sb, slice(0, D), "k"), (wv_sb, slice(0, D), "v")]
                dlist += [(wq_sb, bass.ts(h, D), f"q{h}") for h in range(HEADS)]

                s_tiles = [(ST_A * i, ST_A) for i in range(N_ST_A)]
                n_s = len(s_tiles)

                xt00 = a_xt.tile([128, 2, ST_A], BF16, tag="xt0", name="xt00")
                nc.sync.dma_start(out=xt00[:], in_=xT_t[:, 0:2, 0:ST_A])
                load_w(slice(0, 2))
                xt01 = a_xt.tile([128, 2, ST_A], BF16, tag="xt0", name="xt01")
                nc.sync.dma_start(out=xt01[:], in_=xT_t[:, 2:4, 0:ST_A])

                for st, (s_off, s_w) in enumerate(s_tiles):
                    if st == n_s - 1:
                        # phase-B assets: on the sync queue between the last
                        # two s-tiles' X chunks so queue order delays their
                        # transfers past the earlier X streaming
                        nc.sync.dma_start(
                            out=masks_sb[:],
                            in_=masks[:].rearrange("p (o w) -> p o w", w=QT_W),
                        )
                        nc.sync.dma_start(out=ones_sb[:], in_=ones[:])
                        for j in range(HEADS):
                            nc.sync.dma_start(
                                out=wo_sb[:, j, :], in_=woT_t[:, j, :]
                            )
                    ssl = slice(s_off, s_off + s_w)
                    ps_tiles = {}
                    for _, _, kind in dlist:
                        ps_tiles[kind] = a_ps.tile(
                            [128, ST_A], F32, tag=f"mm_{kind}",
                            name=f"ps_{kind}_{st}",
                        )
                    if st == 0:
                        chunks = [(xt00, 0, 2), (xt01, 2, 4), (None, 4, 8),
                                  (None, 8, 12), (None, 12, 16),
                                  (None, 16, 24), (None, 24, 32)]
                    else:
                        chunks = [(None, c, c + HQ) for c in range(0, HT, HQ)]
                    xts = []
                    for ci, (xt_pre, h0, h1) in enumerate(chunks):
                        if xt_pre is not None:
                            xt = xt_pre
                        else:
                            xt = a_xt.tile([128, h1 - h0, s_w], BF16,
                                           tag="xt", name=f"xt_{st}_{ci}")
                            nc.sync.dma_start(
                                out=xt[:], in_=xT_t[:, h0:h1, ssl],
                            )
                        xts.append((xt, h0, h1))
                        if st == 0:
                            # weight chunks interleave in consumption order
                            w_sched = {2: (2, 4), 4: (4, 8), 8: (8, 12),
                                       12: (12, 16), 16: (16, 24),
                                       24: (24, 32)}
                            if h1 in w_sched:
                                load_w(slice(*w_sched[h1]))
                            elif h1 == HT:
                                # in time for st=0's rope epilogue
                                nc.sync.dma_start(out=cos_sb[:], in_=cosT[:])
                                nc.sync.dma_start(out=sin_sb[:], in_=sinT[:])
                                nc.sync.dma_start(out=ident_sb[:], in_=ident[:])
                        if st < n_s - 1:
                            for w_sb, dsl, kind in dlist:
                                ps = ps_tiles[kind]
                                for ht in range(h1 - h0):
                                    nc.tensor.matmul(
                                        ps[:, 0:s_w],
                                        w_sb[:, h0 + ht, dsl],
                                        xt[:, ht, :],
                                        start=(h0 == 0 and ht == 0),
                                        stop=(h1 == HT and ht == h1 - h0 - 1),
                                    )
                    if st == n_s - 1:
                        # Last s-tile: one output at a time, with its rope
                        # epilogue emitted immediately — each epilogue overlaps
                        # the next output's matmuls, so phase B never waits on
                        # a serial rope tail (and the k/v PSUM banks that B's
                        # score tiles will alias free ~27us before B starts).
                        for w_sb, dsl, kind in dlist:
                            ps = ps_tiles[kind]
                            for xt, h0, h1 in xts:
                                for ht in range(h1 - h0):
                                    nc.tensor.matmul(
                                        ps[:, 0:s_w],
                                        w_sb[:, h0 + ht, dsl],
                                        xt[:, ht, :],
                                        start=(h0 == 0 and ht == 0),
                                        stop=(h1 == HT and ht == h1 - h0 - 1),
                                    )
                            if kind == "k":
                                _rope_epilogue(
                                    nc, a_st, ps[:, 0:s_w], kt_sb[:, ssl],
                                    cos_sb[:, ssl], sin_sb[:, ssl], s_w,
                                )
                            elif kind == "v":
                                nc.vector.tensor_copy(vt_sb[:, ssl],
                                                      ps[:, 0:s_w])
                            else:
                                h = int(kind[1])
                                _rope_epilogue(
                                    nc, a_st, ps[:, 0:s_w], qT_sb[:, h, ssl],
                                    cos_sb[:, ssl], sin_sb[:, ssl], s_w,
                                )
                        # transposes last: their vt dependency cleared long
                        # ago, so PE never stalls on the DVE copy
                        for kj in range(s_w // 128):
                            ki = s_off // 128 + kj
                            tp = a_tps.tile([128, 128], BF16, tag="tp",
                                            name=f"tp_{ki}")
                            nc.tensor.transpose(
                                tp[:], vt_sb[:, bass.ts(ki, 128)], ident_sb[:]
                            )
                            nc.vector.tensor_copy(v_sb[:, ki, :], tp[:])
                        continue
                    # k/v epilogues first: frees their PSUM banks earliest
                    _rope_epilogue(
                        nc, a_st, ps_tiles["k"][:, 0:s_w], kt_sb[:, ssl],
                        cos_sb[:, ssl], sin_sb[:, ssl], s_w,
                    )
                    nc.vector.tensor_copy(vt_sb[:, ssl],
                                          ps_tiles["v"][:, 0:s_w])
                    for h in range(HEADS):
                        _rope_epilogue(
                            nc, a_st, ps_tiles[f"q{h}"][:, 0:s_w],
                            qT_sb[:, h, ssl],
                            cos_sb[:, ssl], sin_sb[:, ssl], s_w,
                        )
                    for kj in range(s_w // 128):
                        ki = s_off // 128 + kj
                        tp = a_tps.tile([128, 128], BF16, tag="tp",
                                        name=f"tp_{ki}")
                        nc.tensor.transpose(
                            tp[:], vt_sb[:, bass.ts(ki, 128)], ident_sb[:]
                        )
                        nc.vector.tensor_copy(v_sb[:, ki, :], tp[:])

            # ---- Phases B+C fused via generators.
            with tc.tile_pool(name="b_p", bufs=6) as b_p, \
                 tc.tile_pool(name="b_da", bufs=3) as b_da, \
                 tc.tile_pool(name="b_r", bufs=2) as b_r, \
                 tc.tile_pool(name="c_row", bufs=2) as c_row, \
                 tc.tile_pool(name="attn", bufs=1) as attn_pool, \
                 tc.tile_pool(name="b_sps", bufs=2, space="PSUM") as b_sps, \
                 tc.tile_pool(name="b_ops", bufs=2, space="PSUM") as b_ops, \
                 tc.tile_pool(name="cd_ps", bufs=2, space="PSUM") as cd_ps:
                attnT_sb = attn_pool.tile([128, HEADS, S], BF16, tag="attnT")

                def b_steps(qi):
                    """Emit phase-B work for q-tile qi; yields at natural
                    C-interleave points."""
                    qsl = bass.ts(qi, QT_W)
                    n_pairs = 2 * qi + 2
                    for hp in range(2):
                        hs = (2 * hp, 2 * hp + 1)
                        ops = {}
                        dens = {}
                        for h in hs:
                            ops[h] = b_ops.tile([128, QT_W], F32, tag="out",
                                                name=f"op_{qi}_{h}")
                            dens[h] = b_da.tile([128, QT_W], BF16, tag="da",
                                                name=f"da_{qi}_{h}")
                        for pk in range(n_pairs):
                            # diagonal k-tiles only need q-columns >= 128*off:
                            # narrow scores/exp/mask/AV/den to the live range
                            is_diag = pk >= n_pairs - 2
                            offs = [
                                2 * (pk - (n_pairs - 2)) + j if is_diag else 0
                                for j in range(2)
                            ]
                            o_j = [128 * max(0, offs[j]) for j in range(2)]
                            p_tiles = {}
                            for h in hs:
                                qt_ap = qT_sb[:, h, qsl]
                                sps = b_sps.tile([128, 2, QT_W], F32, tag="s",
                                                 name=f"s_{qi}_{h}_{pk}")
                                for j in range(2):
                                    ki = 2 * pk + j
                                    nc.tensor.matmul(
                                        sps[:, j, o_j[j] : QT_W],
                                        kt_sb[:, bass.ts(ki, 128)],
                                        qt_ap[:, o_j[j] : QT_W],
                                        start=True, stop=True,
                                    )
                                p_t = b_p.tile([128, 2, QT_W], BF16, tag="p",
                                               name=f"p_{qi}_{h}_{pk}")
                                p_tiles[h] = p_t
                                if is_diag and o_j[1] > 0:
                                    for j in range(2):
                                        nc.scalar.activation(
                                            p_t[:, j, o_j[j] : QT_W],
                                            sps[:, j, o_j[j] : QT_W],
                                            mybir.ActivationFunctionType.Exp,
                                            scale=SM_SCALE,
                                        )
                                else:
                                    nc.scalar.activation(
                                        p_t[:], sps[:],
                                        mybir.ActivationFunctionType.Exp,
                                        scale=SM_SCALE,
                                    )
                                if is_diag:
                                    # only the 128-col triangle boundary of
                                    # each diagonal tile needs masking
                                    for j in range(2):
                                        o = o_j[j]
                                        nc.vector.tensor_mul(
                                            p_t[:, j, o : o + 128],
                                            p_t[:, j, o : o + 128],
                                            masks_sb[:, offs[j], o : o + 128],
                                        )
                            yield  # C fills the exp->AV latency here
                            for h in hs:
                                p_t = p_tiles[h]
                                for j in range(2):
                                    ki = 2 * pk + j
                                    nc.tensor.matmul(
                                        ops[h][:, o_j[j] : QT_W],
                                        v_sb[:, ki, :],
                                        p_t[:, j, o_j[j] : QT_W],
                                        start=(pk == 0 and j == 0),
                                        stop=(pk == n_pairs - 1 and j == 1),
                                        skip_group_check=(o_j[j] > 0),
                                    )
                                if pk == 0:
                                    if o_j[1] > 0:
                                        nc.vector.tensor_copy(
                                            dens[h][:], p_t[:, 0, :]
                                        )
                                        nc.vector.tensor_add(
                                            dens[h][:, o_j[1] :],
                                            dens[h][:, o_j[1] :],
                                            p_t[:, 1, o_j[1] :],
                                        )
                                    else:
                                        nc.vector.tensor_add(
                                            dens[h][:], p_t[:, 0, :],
                                            p_t[:, 1, :],
                                        )
                                else:
                                    for j in range(2):
                                        o = o_j[j]
                                        nc.vector.tensor_add(
                                            dens[h][:, o:], dens[h][:, o:],
                                            p_t[:, j, o:],
                                        )
                            yield
                        for h in hs:
                            den_ps = cd_ps.tile([128, QT_W], F32, tag="cd",
                                                name=f"den_{qi}_{h}")
                            nc.tensor.matmul(
                                den_ps[0:1, :], ones_sb[:, 0:1], dens[h][:],
                                start=True, stop=True,
                            )
                            recip = b_r.tile([1, QT_W], BF16, tag="recip")
                            nc.vector.reciprocal(recip[:], den_ps[0:1, :])
                            yield "norm"  # extra C credit: recip latency
                            bc_ps = cd_ps.tile([128, QT_W], F32, tag="cd",
                                               name=f"bc_{qi}_{h}")
                            nc.tensor.matmul(
                                bc_ps[:], ones_sb[0:1, :], recip[:],
                                start=True, stop=True,
                            )
                            bc_sb = b_r.tile([128, QT_W], F32, tag="bcs")
                            nc.scalar.copy(bc_sb[:], bc_ps[:])
                            nc.vector.tensor_mul(
                                attnT_sb[:, h, qsl], ops[h][:], bc_sb[:]
                            )
                            yield

                def c_steps(cqi):
                    """Output projection for q-tile cqi: 4 si x 8 ei groups,
                    staged into [128, H] rows. PSUM->SBUF copies alternate
                    ACT/DVE to keep ACT headroom for the exp chain."""
                    last_si = cqi == N_QT - 1
                    for sj in range(QT_W // 128):
                        si = cqi * (QT_W // 128) + sj
                        tail = last_si and sj == QT_W // 128 - 1
                        o_row = c_row.tile([128, H // ET, ET], BF16, tag="orow",
                                           name=f"orow_{si}")
                        n_e = H // ET
                        for ei in range(n_e):
                            o_ps = cd_ps.tile([128, ET], F32, tag="cd",
                                              name=f"o_{si}_{ei}")
                            for j in range(HEADS):
                                nc.tensor.matmul(
                                    o_ps[:],
                                    attnT_sb[:, j, bass.ts(si, 128)],
                                    wo_sb[:, j, bass.ts(ei, ET)],
                                    start=(j == 0),
                                    stop=(j == HEADS - 1),
                                )
                            if ei % 2 == 0:
                                nc.scalar.copy(o_row[:, ei, :], o_ps[:])
                            else:
                                nc.vector.tensor_copy(o_row[:, ei, :], o_ps[:])
                            # stream the row out: on the tail group, one DMA
                            # per copy spread over queues (shortest drain);
                            # halves otherwise
                            step = 1 if tail else 4
                            if (ei + 1) % step == 0:
                                e0 = ei + 1 - step
                                eng = (
                                    [nc.gpsimd, nc.sync][ei % 2]
                                    if tail else nc.gpsimd
                                )
                                eng.dma_start(
                                    out=out[:][
                                        bass.ts(si, 128), e0 * ET : (ei + 1) * ET
                                    ],
                                    in_=o_row[:, e0 : ei + 1, :].rearrange(
                                        "p e w -> p (e w)"
                                    ),
                                )
                            yield

                def drive(bgen, cgen, rate):
                    credit = 0.0
                    for tag_ in bgen:
                        credit += rate + (1.0 if tag_ == "norm" else 0.0)
                        while credit >= 1.0:
                            credit -= 1.0
                            if next(cgen, None) is None:
                                credit = 0.0
                                break

                pending_c = iter(())
                for qi in range(N_QT):
                    # b yields per qi: 4 * n_pairs + 8; c steps pending: 32
                    n_b = 4 * (2 * qi + 2) + 8
                    rate = 32.0 / n_b if qi > 0 else 0.0
                    drive(b_steps(qi), pending_c, rate)
                    # drain any leftover C steps for qi-1
                    for _ in pending_c:
                        pass
                    pending_c = c_steps(qi)
                # tail: C for the last q-tile
                for _ in pending_c:
                    pass
    _split_sync_waits(nc)
    return nc


_NC_CACHE = None


def _get_program():
    global _NC_CACHE
    if _NC_CACHE is None:
        _NC_CACHE = _build_program()
    return _NC_CACHE


def _host_tables(position_ids):
    pos = position_ids.reshape(-1).astype(np.float32)  # [S]
    inv_freq = (
        1.0
        / (np.float32(ROPE_THETA) ** (np.arange(0, D, 2, dtype=np.float32) / np.float32(D)))
    ).astype(np.float32)  # [64]
    freqs = pos[None, :] * inv_freq[:, None]  # [64, S]
    ang = np.concatenate([freqs, freqs], axis=0)  # [128, S]
    cosT = np.cos(ang).astype(np.float32)
    sinT = np.sin(ang).astype(np.float32)
    sinT[0:64, :] *= -1.0  # sign-fold for rotate_half

    masks = np.zeros((128, 4 * QT_W), dtype=np.float32)
    for off in range(4):
        p = np.arange(128)[:, None]
        c = np.arange(QT_W)[None, :]
        masks[:, off * QT_W : (off + 1) * QT_W] = (128 * off + p <= c).astype(
            np.float32
        )
    return cosT, sinT, masks


def _prepare_in_maps(hidden_states, Wq, Wk, Wv, Wo, position_ids):
    bf16 = mybir.dt.np(mybir.dt.bfloat16)
    x = np.asarray(hidden_states, dtype=np.float32).reshape(S, H)
    Wq = np.asarray(Wq, dtype=np.float32)
    Wk = np.asarray(Wk, dtype=np.float32)
    Wv = np.asarray(Wv, dtype=np.float32)
    Wo = np.asarray(Wo, dtype=np.float32)

    xT = np.ascontiguousarray(x.T).astype(bf16)  # [H, S]
    cosT, sinT, masks = _host_tables(np.asarray(position_ids))
    cosT = cosT.astype(bf16)
    sinT = sinT.astype(bf16)
    masks = masks.astype(bf16)
    ident = np.eye(128, dtype=np.float32).astype(bf16)
    ones = np.ones((128, 128), dtype=np.float32).astype(bf16)

    in_maps = []
    for c in range(N_CORES):
        qs = slice(DQ * c, DQ * (c + 1))
        ks = slice(DKV * c, DKV * (c + 1))
        in_maps.append(
            {
                "xT": xT,
                "wqT": np.ascontiguousarray(Wq[qs, :].T).astype(bf16),
                "wkT": np.ascontiguousarray(Wk[ks, :].T).astype(bf16),
                "wvT": np.ascontiguousarray(Wv[ks, :].T).astype(bf16),
                "woT": np.ascontiguousarray(Wo[:, qs].T).astype(bf16),
                "cosT": cosT,
                "sinT": sinT,
                "masks": masks,
                "ident": ident,
                "ones": ones,
            }
        )
    return in_maps


def _finalize(results, batch):
    out = np.zeros((S, H), dtype=np.float32)
    for c in range(N_CORES):
        out += results[c]["out"].astype(np.float32)
    return out.reshape(batch, S, H)


def kernel(hidden_states, Wq, Wk, Wv, Wo, position_ids):
    from concourse.bass_utils import run_bass_kernel_spmd

    B = hidden_states.shape[0]
    in_maps = _prepare_in_maps(hidden_states, Wq, Wk, Wv, Wo, position_ids)
    nc = _get_program()
    res = run_bass_kernel_spmd(nc, in_maps, list(range(N_CORES)))
    return _finalize(res.results, B)
